# revision 1
# baseline (speedup 1.0000x reference)
"""DeformConv2d (DCNv2) Trainium2 Bass kernel.

Problem: N=4, C_IN=C_OUT=64, H=W=128, 3x3 taps, stride=1, pad=1, dil=1,
modulated deformable conv (torchvision semantics).

Sharding: 8 cores; core = (image n = core//2, row-half = core%2).
Each core computes out[n, :, i0:i0+64, :] from the full image x[n].

Per-core pipeline (all arithmetic on device):
  1. DVE: frac/floor of offsets via AluOpType.mod, bilinear corner weights
     (modulation mask folded in), int16 gather indices built against a
     constant affine index ramp.
  2. Pool/SWDGE: dma_gather pair-mode from zero-padded NHWC image in DRAM
     (elem = 2 pixels x 64ch x fp32 = 512B per descriptor; y0/y1 rows are
     two descriptors).  Zero padding makes out-of-bounds corners exact.
  3. DVE: weighted 4-corner combine using stride-0 broadcast weight APs.
  4. PE: per-row transposes [128j, 64c] -> [64c, 128j], then 9 accumulating
     matmuls (contraction c=64) with weight slices as stationaries.
"""
import sys
import os

_TRN_REPO = "/opt/trn_rl_repo"
if _TRN_REPO not in sys.path:
    sys.path.insert(0, _TRN_REPO)

import numpy as np

import concourse.bass as bass
import concourse.bacc as bacc
import concourse.tile as tile
import concourse.mybir as mybir
from concourse import library_config
from concourse.bass_utils import run_bass_kernel_spmd
from contextlib import ExitStack

F32 = mybir.dt.float32
I16 = mybir.dt.int16
ALU = mybir.AluOpType

N, C, H, W = 4, 64, 128, 128
K2 = 9
PAD = 16                    # coordinate padding on each side
PH = H + 2 * PAD            # 160
PW = W + 2 * PAD            # 160
NROWS = PH * PW             # 25600 pixel-rows of 64 channels in padded image
HI = 64                     # rows per core
R = 16                      # rows per block
NBLK = HI // R              # 4
NIDX_BLK = R * 2 * W        # 4096 gather descriptors per (k, block)
CLAMP = 11.0                # |floor(offset)| clamp (pad-region safe)

_CACHED = {}


def build_nc():
    nc = bacc.Bacc(trn_type="TRN2", debug=False, num_swdge_queues=4)

    xp_d = nc.dram_tensor("xp", [NROWS * C], F32, kind="ExternalInput")
    offj_d = nc.dram_tensor("offj", [128, 2 * K2 * HI], F32, kind="ExternalInput").ap()
    maskj_d = nc.dram_tensor("maskj", [128, K2 * HI], F32, kind="ExternalInput").ap()
    idxb_d = nc.dram_tensor("idxb", [128, K2 * HI * 2 * 8], F32, kind="ExternalInput").ap()
    wk_d = nc.dram_tensor("wk", [64, K2 * 64], F32, kind="ExternalInput").ap()
    ident_d = nc.dram_tensor("ident", [128, 128], F32, kind="ExternalInput").ap()
    out_d = nc.dram_tensor("out", [64, HI * W], F32, kind="ExternalOutput").ap()
    scr_d = nc.dram_tensor("dyx_scratch", [128 * K2 * HI], F32)

    # gather source: overlapping pixel-pair rows of the padded image
    src_ap = bass.AP(xp_d, 0, [[C, NROWS - 1], [1, 2 * C]])

    with ExitStack() as ctx:
        tc = ctx.enter_context(tile.TileContext(nc))

        const = ctx.enter_context(tc.tile_pool(name="const", bufs=1))
        # live across phase 2: idxs + w4
        live = ctx.enter_context(tc.tile_pool(name="live", bufs=1))
        scratch_ctx = ExitStack()
        work = scratch_ctx.enter_context(tc.tile_pool(name="work", bufs=1))

        offj = const.tile([128, 2 * K2 * HI], F32)
        nc.sync.dma_start(offj[:], offj_d)
        maskj = const.tile([128, K2 * HI], F32)
        nc.sync.dma_start(maskj[:], maskj_d)
        idxb = const.tile([128, K2 * HI * 2 * 8], F32)
        nc.sync.dma_start(idxb[:], idxb_d)
        wk = const.tile([64, K2 * 64], F32)
        nc.sync.dma_start(wk[:], wk_d)
        ident = const.tile([128, 128], F32)
        nc.sync.dma_start(ident[:], ident_d)

        # ---- Phase 1: frac / floor / weights / indices -------------------
        # floor via round-to-nearest magic constant: rne(x) = (x + M) - M,
        # floor(x) = rne(x) - (rne(x) > x); frac = x - floor(x).  Exact for
        # |x| < 2^22 in fp32; avoids AluOpType.mod (not in DVE ISA).
        MAGIC = 12582912.0  # 1.5 * 2**23
        flo = work.tile([128, 2 * K2 * HI], F32)
        nc.vector.tensor_scalar(flo[:], offj[:], MAGIC, None, ALU.add)
        nc.vector.tensor_scalar(flo[:], flo[:], MAGIC, None, ALU.subtract)
        rup = work.tile([128, 2 * K2 * HI], F32)
        nc.vector.tensor_tensor(rup[:], flo[:], offj[:], ALU.is_gt)
        nc.vector.tensor_tensor(flo[:], flo[:], rup[:], ALU.subtract)
        frac = work.tile([128, 2 * K2 * HI], F32)
        nc.vector.tensor_tensor(frac[:], offj[:], flo[:], ALU.subtract)
        nc.vector.tensor_scalar(flo[:], flo[:], -CLAMP, None, ALU.max)
        nc.vector.tensor_scalar(flo[:], flo[:], CLAMP, None, ALU.min)

        # offj channel layout: ch = 2k (dy), 2k+1 (dx); free = (ch, i)
        def kv(t):  # [128, (k, two, i)]
            return t[:].rearrange("p (k two i) -> p k two i", k=K2, two=2, i=HI)

        dyx = work.tile([128, K2 * HI], F32)
        dyx3 = dyx[:].rearrange("p (k i) -> p k i", k=K2, i=HI)
        nc.vector.tensor_scalar(dyx3, kv(flo)[:, :, 0, :], float(PW), None, ALU.mult)
        nc.vector.tensor_tensor(dyx3, dyx3, kv(flo)[:, :, 1, :], ALU.add)

        # repack dyx [j, (k,i)] -> dyx_w [16q+u, (k,i,jw)] via DRAM bounce
        nc.sync.dma_start(bass.AP(scr_d, 0, [[K2 * HI, 128], [1, K2 * HI]]), dyx[:])
        dyx_w = work.tile([128, K2 * HI * 8], F32)
        for q in range(8):
            # dst partition 16q+u, free (k, i, jw); src scratch[(16*jw+u)*576 + k*64 + i]
            nc.sync.dma_start(
                dyx_w[16 * q:16 * q + 16, :].rearrange(
                    "p (k i jw) -> p k i jw", k=K2, i=HI, jw=8),
                bass.AP(scr_d, 0,
                        [[K2 * HI, 16], [64, K2], [1, HI], [16 * K2 * HI, 8]]),
            )

        # idxs[p, (k, i, yc, jw)] = idxb + dyx_w (broadcast over yc)
        idxs = live.tile([128, K2 * HI * 2 * 8], I16)
        dyx_b = bass.AP(
            dyx_w[:].tensor, dyx_w[:].offset,
            [dyx_w[:].ap[0], [8, K2 * HI], [0, 2], [1, 8]],
        )
        nc.vector.tensor_tensor(
            idxs[:].rearrange("p (ki yc jw) -> p ki yc jw",
                              ki=K2 * HI, yc=2, jw=8),
            idxb[:].rearrange("p (ki yc jw) -> p ki yc jw",
                              ki=K2 * HI, yc=2, jw=8),
            dyx_b, ALU.add)

        # corner weights w4[j, (k, i, yc, xc)]
        fr = kv(frac)
        wy = fr[:, :, 0, :]            # [128, k, i]
        wx = fr[:, :, 1, :]
        omy = work.tile([128, K2 * HI], F32)
        omyv = omy[:].rearrange("p (k i) -> p k i", k=K2, i=HI)
        nc.vector.tensor_scalar(omyv, wy, 1.0, None, ALU.subtract)
        nc.vector.tensor_scalar(omyv, omyv, -1.0, None, ALU.mult)
        omx = work.tile([128, K2 * HI], F32)
        omxv = omx[:].rearrange("p (k i) -> p k i", k=K2, i=HI)
        nc.vector.tensor_scalar(omxv, wx, 1.0, None, ALU.subtract)
        nc.vector.tensor_scalar(omxv, omxv, -1.0, None, ALU.mult)
        m3 = maskj[:].rearrange("p (k i) -> p k i", k=K2, i=HI)
        wxm0 = work.tile([128, K2 * HI], F32)
        nc.vector.tensor_tensor(
            wxm0[:].rearrange("p (k i) -> p k i", k=K2, i=HI),
            omx[:].rearrange("p (k i) -> p k i", k=K2, i=HI), m3, ALU.mult)
        wxm1 = work.tile([128, K2 * HI], F32)
        nc.vector.tensor_tensor(
            wxm1[:].rearrange("p (k i) -> p k i", k=K2, i=HI), wx, m3, ALU.mult)

        w4 = live.tile([128, K2 * HI * 4], F32)
        w4v = w4[:].rearrange("p (k i yc xc) -> p k i yc xc",
                              k=K2, i=HI, yc=2, xc=2)
        omy3 = omy[:].rearrange("p (k i) -> p k i", k=K2, i=HI)
        wy3 = wy
        wxm0v = wxm0[:].rearrange("p (k i) -> p k i", k=K2, i=HI)
        wxm1v = wxm1[:].rearrange("p (k i) -> p k i", k=K2, i=HI)
        nc.vector.tensor_tensor(w4v[:, :, :, 0, 0], omy3, wxm0v, ALU.mult)
        nc.vector.tensor_tensor(w4v[:, :, :, 0, 1], omy3, wxm1v, ALU.mult)
        nc.vector.tensor_tensor(w4v[:, :, :, 1, 0], wy3, wxm0v, ALU.mult)
        nc.vector.tensor_tensor(w4v[:, :, :, 1, 1], wy3, wxm1v, ALU.mult)

        # ---- Phase 2: gather / combine / transpose / conv ----------------
        scratch_ctx.close()
        gpool = ctx.enter_context(tc.tile_pool(name="g", bufs=2))
        p4pool = ctx.enter_context(tc.tile_pool(name="p4", bufs=2))
        s2pool = ctx.enter_context(tc.tile_pool(name="s2", bufs=2))
        spool = ctx.enter_context(tc.tile_pool(name="s", bufs=2))
        stpool = ctx.enter_context(tc.tile_pool(name="st", bufs=2))
        obpool = ctx.enter_context(tc.tile_pool(name="ob", bufs=2))
        tpps = ctx.enter_context(tc.tile_pool(name="tp", bufs=2, space="PSUM"))
        outps = ctx.enter_context(tc.tile_pool(name="ops", bufs=1, space="PSUM"))

        idxs5 = idxs[:].rearrange("p (k i yc jw) -> p k i yc jw",
                                  k=K2, i=HI, yc=2, jw=8)
        w4_5 = w4[:].rearrange("p (k i yc xc) -> p k i yc xc",
                               k=K2, i=HI, yc=2, xc=2)

        for b in range(NBLK):
            out_ps = outps.tile([64, R * W], F32)
            for k in range(K2):
                g = gpool.tile([128, R * 2 * 128], F32)
                # SWDGE ring holds 1024 descriptors -> 4 rows (1024 idxs)
                # per dma_gather call, round-robined over 4 queues.
                RSUB = 4
                gv = g[:].rearrange("p (s e) -> p s e", s=R * 2, e=128)
                for sub in range(R // RSUB):
                    nidx = RSUB * 2 * 128
                    nc.gpsimd.dma_gather(
                        gv[:, sub * RSUB * 2:(sub + 1) * RSUB * 2, :],
                        src_ap,
                        idxs5[:, k, b * R + sub * RSUB:b * R + (sub + 1) * RSUB, :, :],
                        nidx,
                        nidx,
                        elem_size=2 * C,
                        elem_step=C,
                        queue_num=(b * K2 * (R // RSUB) + k * (R // RSUB) + sub) % 4,
                    )
                # weighted corners
                p4 = p4pool.tile([128, R * 2 * 2 * C], F32)
                wsl = w4_5[:, k, b * R:(b + 1) * R, :, :]
                w_b = bass.AP(
                    wsl.tensor, wsl.offset,
                    [wsl.ap[0], [4, R], [1, 4], [0, C]],
                )
                nc.vector.tensor_tensor(
                    p4[:].rearrange("p (i cr c) -> p i cr c", i=R, cr=4, c=C),
                    g[:].rearrange("p (i cr c) -> p i cr c", i=R, cr=4, c=C),
                    w_b, ALU.mult)
                # sum y-corners, then x-corners
                s2 = s2pool.tile([128, R * 2 * C], F32)
                p4v = p4[:].rearrange("p (i yc cc) -> p i yc cc",
                                      i=R, yc=2, cc=2 * C)
                nc.vector.tensor_tensor(
                    s2[:].rearrange("p (i cc) -> p i cc", i=R, cc=2 * C),
                    p4v[:, :, 0, :], p4v[:, :, 1, :], ALU.add)
                s = spool.tile([128, R * C], F32)
                s2v = s2[:].rearrange("p (i xc c) -> p i xc c", i=R, xc=2, c=C)
                sv = s[:].rearrange("p (i c) -> p i c", i=R, c=C)
                nc.vector.tensor_tensor(
                    sv, s2v[:, :, 0, :], s2v[:, :, 1, :], ALU.add)
                # transpose to [c, (i, j)] and conv-accumulate
                st = stpool.tile([64, R * 128], F32)
                for h in range(R // 8):
                    tp = tpps.tile([64, 8 * 128], F32)
                    for i2 in range(8):
                        i = h * 8 + i2
                        nc.tensor.transpose(
                            tp[:, i2 * 128:(i2 + 1) * 128], sv[:, i, :], ident[:])
                    nc.scalar.copy(
                        st[:, h * 8 * 128:(h + 1) * 8 * 128], tp[:])
                for c4 in range(R * W // 512):
                    nc.tensor.matmul(
                        out_ps[:, c4 * 512:(c4 + 1) * 512],
                        wk[:, k * 64:(k + 1) * 64],
                        st[:, c4 * 512:(c4 + 1) * 512],
                        start=(k == 0), stop=(k == K2 - 1))
            ob = obpool.tile([64, R * W], F32)
            nc.scalar.copy(ob[:], out_ps[:])
            nc.sync.dma_start(out_d[:, b * R * W:(b + 1) * R * W], ob[:])

    if not nc.is_finalized():
        nc.finalize()
    return nc


def _prep_core(x, offset, mask, weight_kco, core):
    n, half = core // 2, core % 2
    i0 = half * HI
    xp = np.zeros((PH, PW, C), np.float32)
    xp[PAD:PAD + H, PAD:PAD + W, :] = x[n].transpose(1, 2, 0)
    offj = np.ascontiguousarray(
        offset[n, :, i0:i0 + HI, :].transpose(2, 0, 1)).reshape(128, 2 * K2 * HI)
    maskj = np.ascontiguousarray(
        mask[n, :, i0:i0 + HI, :].transpose(2, 0, 1)).reshape(128, K2 * HI)

    u = (np.arange(128) % 16).astype(np.int64)
    k = np.arange(K2)
    ki, kj = k // 3, k % 3
    i = np.arange(HI)
    yc = np.arange(2)
    jw = np.arange(8)
    base = ((i0 + i[None, :, None, None] + ki[:, None, None, None] - 1 + PAD
             + yc[None, None, :, None]) * PW
            + jw[None, None, None, :] * 16 + kj[:, None, None, None] - 1 + PAD)
    idxb = (base[None] + u[:, None, None, None, None]).reshape(128, -1)
    assert idxb.min() >= -CLAMP * PW - CLAMP and idxb.max() + CLAMP * PW + CLAMP + PW < NROWS
    idxb = idxb.astype(np.float32)

    return {
        "xp": xp.reshape(-1),
        "offj": offj,
        "maskj": maskj,
        "idxb": idxb,
        "wk": weight_kco,
        "ident": np.eye(128, dtype=np.float32),
    }


def kernel_traced(x, offset, mask, weight, trace=True, trace_kwargs=None):
    """Like kernel() but runs with NTFF tracing; returns (out, BassKernelResults)."""
    x = np.asarray(x, np.float32)
    offset = np.asarray(offset, np.float32)
    mask = np.asarray(mask, np.float32)
    weight = np.asarray(weight, np.float32)
    weight_kco = np.ascontiguousarray(
        weight.reshape(C, C, K2).transpose(1, 2, 0)).reshape(C, K2 * C)
    if "nc" not in _CACHED:
        _CACHED["nc"] = build_nc()
    nc = _CACHED["nc"]
    in_maps = [
        _prep_core(x, offset, mask, weight_kco, core) for core in range(8)
    ]
    res = run_bass_kernel_spmd(nc, in_maps, list(range(8)), trace=trace,
                               **(trace_kwargs or {}))
    out = np.empty((N, C, H, W), np.float32)
    for core in range(8):
        n, half = core // 2, core % 2
        out[n, :, half * HI:(half + 1) * HI, :] = (
            res.results[core]["out"].reshape(C, HI, W))
    return out, res


def kernel(x, offset, mask, weight):
    x = np.asarray(x, np.float32)
    offset = np.asarray(offset, np.float32)
    mask = np.asarray(mask, np.float32)
    weight = np.asarray(weight, np.float32)
    # wk[c, (k, o)] = weight[o, c, ki, kj]
    weight_kco = np.ascontiguousarray(
        weight.reshape(C, C, K2).transpose(1, 2, 0)).reshape(C, K2 * C)

    if "nc" not in _CACHED:
        _CACHED["nc"] = build_nc()
    nc = _CACHED["nc"]

    in_maps = [
        _prep_core(x, offset, mask, weight_kco, core) for core in range(8)
    ]
    res = run_bass_kernel_spmd(nc, in_maps, list(range(8)))
    out = np.empty((N, C, H, W), np.float32)
    for core in range(8):
        n, half = core // 2, core % 2
        out[n, :, half * HI:(half + 1) * HI, :] = (
            res.results[core]["out"].reshape(C, HI, W))
    return out



# revision 4
# speedup vs baseline: 2.8566x; 2.8566x over previous
"""DeformConv2d (DCNv2) Trainium2 Bass kernel.

Problem: N=4, C_IN=C_OUT=64, H=W=128, 3x3 taps, stride=1, pad=1, dil=1,
modulated deformable conv (torchvision semantics).

Sharding: 8 cores; core = (image n = core//2, row-half = core%2).
Each core computes out[n, :, i0:i0+64, :] from the full image x[n].

Per-core pipeline (all arithmetic on device):
  1. DVE: frac/floor of offsets, bilinear corner weights (modulation mask
     folded in, bf16), int16 gather indices.  The j term is folded into the
     floored displacement via a per-partition tensor_scalar; the affine tap
     base A(k,i) comes from a small replicated table, so the index build is
     one fused add over [128, 4608].
  2. Pool/SWDGE: dma_gather from a precombined 4-corner bf16 table in DRAM:
     R4[y, x] = [(y,x), (y,x+1), (y+1,x), (y+1,x+1)] x 64ch = 512B.  One
     descriptor fetches all four bilinear corners of one (tap, out-pixel).
  3. DVE: weighted 4-corner combine in bf16 (2 elem/cycle).
  4. PE: per-row transposes [128j, 64c] -> [64c, 128j] (bf16), taps paired
     two-per-matmul for a full 128-deep contraction; 5 accumulating matmul
     groups per row block.
"""
import sys
import os

_TRN_REPO = "/opt/trn_rl_repo"
if _TRN_REPO not in sys.path:
    sys.path.insert(0, _TRN_REPO)

import numpy as np
import ml_dtypes

import concourse.bass as bass
import concourse.bacc as bacc
import concourse.tile as tile
import concourse.mybir as mybir
from concourse.bass_utils import run_bass_kernel_spmd
from contextlib import ExitStack

F32 = mybir.dt.float32
BF16 = mybir.dt.bfloat16
I16 = mybir.dt.int16
ALU = mybir.AluOpType
NPBF16 = ml_dtypes.bfloat16

N, C, H, W = 4, 64, 128, 128
K2 = 9
PAD = 16                    # coordinate padding on each side
PH = H + 2 * PAD            # 160
PW = W + 2 * PAD            # 160
NENT = PH * PW              # 25600 R4 entries (4 corners x 64ch each)
HI = 64                     # rows per core
R = 16                      # rows per block
NBLK = HI // R              # 4
CLAMP = 11.0                # |floor(offset)| clamp (pad-region safe)

_CACHED = {}


def build_nc():
    nc = bacc.Bacc(trn_type="TRN2", debug=False, num_swdge_queues=4)

    r4_d = nc.dram_tensor("r4", [NENT * 4 * C], BF16, kind="ExternalInput")
    offj_d = nc.dram_tensor("offj", [128, 2 * K2 * HI], F32, kind="ExternalInput").ap()
    maskj_d = nc.dram_tensor("maskj", [128, K2 * HI], F32, kind="ExternalInput").ap()
    base_d = nc.dram_tensor("base", [128, K2 * HI], F32, kind="ExternalInput").ap()
    j128_d = nc.dram_tensor("j128", [128, 1], F32, kind="ExternalInput").ap()
    wk2_d = nc.dram_tensor("wk2", [128, 4 * 64], BF16, kind="ExternalInput").ap()
    wkl_d = nc.dram_tensor("wkl", [64, 64], BF16, kind="ExternalInput").ap()
    ident_d = nc.dram_tensor("ident", [128, 128], BF16, kind="ExternalInput").ap()
    out_d = nc.dram_tensor("out", [64, HI * W], F32, kind="ExternalOutput").ap()
    scr_d = nc.dram_tensor("dyx_scratch", [128 * K2 * HI], F32)

    # gather source: one 512B entry = 4 bilinear corners x 64ch bf16
    src_ap = bass.AP(r4_d, 0, [[4 * C, NENT - 1], [1, 4 * C]])

    with ExitStack() as ctx:
        tc = ctx.enter_context(tile.TileContext(nc))

        const = ctx.enter_context(tc.tile_pool(name="const", bufs=1))
        live = ctx.enter_context(tc.tile_pool(name="live", bufs=1))
        scratch_ctx = ExitStack()
        work = scratch_ctx.enter_context(tc.tile_pool(name="work", bufs=1))

        offj = const.tile([128, 2 * K2 * HI], F32)
        nc.sync.dma_start(offj[:], offj_d)
        maskj = const.tile([128, K2 * HI], F32)
        nc.sync.dma_start(maskj[:], maskj_d)
        base = const.tile([128, K2 * HI], F32)
        nc.sync.dma_start(base[:], base_d)
        j128 = const.tile([128, 1], F32)
        nc.sync.dma_start(j128[:], j128_d)
        wk2 = const.tile([128, 4 * 64], BF16)
        nc.sync.dma_start(wk2[:], wk2_d)
        wkl = const.tile([64, 64], BF16)
        nc.sync.dma_start(wkl[:], wkl_d)
        ident = const.tile([128, 128], BF16)
        nc.sync.dma_start(ident[:], ident_d)

        # ---- Phase 1: frac / floor / weights / indices -------------------
        # floor via round-to-nearest magic constant: rne(x) = (x + M) - M,
        # floor(x) = rne(x) - (rne(x) > x); frac = x - floor(x).  Exact for
        # |x| < 2^22 in fp32.
        MAGIC = 12582912.0  # 1.5 * 2**23
        flo = work.tile([128, 2 * K2 * HI], F32)
        nc.vector.tensor_scalar(flo[:], offj[:], MAGIC, None, ALU.add)
        nc.vector.tensor_scalar(flo[:], flo[:], MAGIC, None, ALU.subtract)
        rup = work.tile([128, 2 * K2 * HI], F32)
        nc.vector.tensor_tensor(rup[:], flo[:], offj[:], ALU.is_gt)
        nc.vector.tensor_tensor(flo[:], flo[:], rup[:], ALU.subtract)
        frac = work.tile([128, 2 * K2 * HI], F32)
        nc.vector.tensor_tensor(frac[:], offj[:], flo[:], ALU.subtract)
        nc.vector.tensor_scalar(flo[:], flo[:], -CLAMP, None, ALU.max)
        nc.vector.tensor_scalar(flo[:], flo[:], CLAMP, None, ALU.min)

        # offj channel layout: ch = 2k (dy), 2k+1 (dx); free = (ch, i)
        def kv(t):  # [128, (k, two, i)]
            return t[:].rearrange("p (k two i) -> p k two i", k=K2, two=2, i=HI)

        # dyx[j, (k,i)] = floor_dy*PW + floor_dx + j   (j folded in here)
        dyx = work.tile([128, K2 * HI], F32)
        dyx3 = dyx[:].rearrange("p (k i) -> p k i", k=K2, i=HI)
        nc.vector.tensor_scalar(dyx3, kv(flo)[:, :, 0, :], float(PW), j128[:],
                                ALU.mult, ALU.add)
        nc.vector.tensor_tensor(dyx3, dyx3, kv(flo)[:, :, 1, :], ALU.add)

        # repack dyx [j, (k,i)] -> dyx_w [16q+u, (jw,k,i)] via DRAM bounce.
        # dst free order (jw,k,i) keeps 2304B-contiguous runs on both sides.
        nc.sync.dma_start(bass.AP(scr_d, 0, [[K2 * HI, 128], [1, K2 * HI]]), dyx[:])
        dyx_w = work.tile([128, 8 * K2 * HI], F32)
        for q in range(8):
            nc.sync.dma_start(
                dyx_w[16 * q:16 * q + 16, :].rearrange(
                    "p (jw k i) -> p jw k i", jw=8, k=K2, i=HI),
                bass.AP(scr_d, 0,
                        [[K2 * HI, 16], [16 * K2 * HI, 8], [HI, K2], [1, HI]]),
            )

        # idxs[p, (k,i,jw)] = base(k,i) + dyx_w  (single fused add -> int16)
        idxs = live.tile([128, K2 * HI * 8], I16)
        dw = dyx_w[:]
        dyx_v = bass.AP(
            dw.tensor, dw.offset,
            [dw.ap[0], [HI, K2], [1, HI], [K2 * HI, 8]],
        )
        bs = base[:]
        base_v = bass.AP(
            bs.tensor, bs.offset,
            [bs.ap[0], [HI, K2], [1, HI], [0, 8]],
        )
        nc.vector.tensor_tensor(
            idxs[:].rearrange("p (k i jw) -> p k i jw", k=K2, i=HI, jw=8),
            dyx_v, base_v, ALU.add)

        # corner weights w4[j, (k, i, yc, xc)] in bf16, mask folded in
        fr = kv(frac)
        wy = fr[:, :, 0, :]            # [128, k, i]
        wx = fr[:, :, 1, :]
        omy = work.tile([128, K2 * HI], F32)
        omyv = omy[:].rearrange("p (k i) -> p k i", k=K2, i=HI)
        nc.vector.tensor_scalar(omyv, wy, 1.0, -1.0, ALU.subtract, ALU.mult)
        omx = work.tile([128, K2 * HI], F32)
        omxv = omx[:].rearrange("p (k i) -> p k i", k=K2, i=HI)
        nc.vector.tensor_scalar(omxv, wx, 1.0, -1.0, ALU.subtract, ALU.mult)
        m3 = maskj[:].rearrange("p (k i) -> p k i", k=K2, i=HI)
        wxm0 = work.tile([128, K2 * HI], F32)
        nc.vector.tensor_tensor(
            wxm0[:].rearrange("p (k i) -> p k i", k=K2, i=HI), omxv, m3, ALU.mult)
        wxm1 = work.tile([128, K2 * HI], F32)
        nc.vector.tensor_tensor(
            wxm1[:].rearrange("p (k i) -> p k i", k=K2, i=HI), wx, m3, ALU.mult)

        w4 = live.tile([128, K2 * HI * 4], BF16)
        w4v = w4[:].rearrange("p (k i yc xc) -> p k i yc xc",
                              k=K2, i=HI, yc=2, xc=2)
        wxm0v = wxm0[:].rearrange("p (k i) -> p k i", k=K2, i=HI)
        wxm1v = wxm1[:].rearrange("p (k i) -> p k i", k=K2, i=HI)
        nc.vector.tensor_tensor(w4v[:, :, :, 0, 0], omyv, wxm0v, ALU.mult)
        nc.vector.tensor_tensor(w4v[:, :, :, 0, 1], omyv, wxm1v, ALU.mult)
        nc.vector.tensor_tensor(w4v[:, :, :, 1, 0], wy, wxm0v, ALU.mult)
        nc.vector.tensor_tensor(w4v[:, :, :, 1, 1], wy, wxm1v, ALU.mult)

        # ---- Phase 2: gather / combine / transpose / conv ----------------
        scratch_ctx.close()
        gpool = ctx.enter_context(tc.tile_pool(name="g", bufs=3))
        p4pool = ctx.enter_context(tc.tile_pool(name="p4", bufs=2))
        s2pool = ctx.enter_context(tc.tile_pool(name="s2", bufs=2))
        spool = ctx.enter_context(tc.tile_pool(name="s", bufs=2))
        stpool = ctx.enter_context(tc.tile_pool(name="st", bufs=2))
        obpool = ctx.enter_context(tc.tile_pool(name="ob", bufs=2))
        tpps = ctx.enter_context(tc.tile_pool(name="tp", bufs=2, space="PSUM"))
        outps = ctx.enter_context(tc.tile_pool(name="ops", bufs=1, space="PSUM"))

        idxs4 = idxs[:].rearrange("p (k i jw) -> p k i jw", k=K2, i=HI, jw=8)
        w4_5 = w4[:].rearrange("p (k i yc xc) -> p k i yc xc",
                               k=K2, i=HI, yc=2, xc=2)

        qn = [0]
        for b in range(NBLK):
            out_ps = outps.tile([64, R * W], F32)
            st2 = None
            for k in range(K2):
                g = gpool.tile([128, R * 4 * C], BF16)
                # SWDGE ring holds 1024 descriptors -> 8 rows (1024 idxs)
                # per dma_gather call, round-robined over 4 queues.
                gv = g[:].rearrange("p (s e) -> p s e", s=R, e=4 * C)
                for sub in range(2):
                    nidx = 8 * 128
                    nc.gpsimd.dma_gather(
                        gv[:, sub * 8:(sub + 1) * 8, :],
                        src_ap,
                        idxs4[:, k, b * R + sub * 8:b * R + (sub + 1) * 8, :],
                        nidx,
                        nidx,
                        elem_size=4 * C,
                        elem_step=4 * C,
                        queue_num=qn[0] % 4,
                    )
                    qn[0] += 1
                # weighted corners (bf16)
                p4 = p4pool.tile([128, R * 4 * C], BF16)
                wsl = w4_5[:, k, b * R:(b + 1) * R, :, :]
                w_b = bass.AP(
                    wsl.tensor, wsl.offset,
                    [wsl.ap[0], [4, R], [1, 4], [0, C]],
                )
                nc.vector.tensor_tensor(
                    p4[:].rearrange("p (i cr c) -> p i cr c", i=R, cr=4, c=C),
                    g[:].rearrange("p (i cr c) -> p i cr c", i=R, cr=4, c=C),
                    w_b, ALU.mult)
                # sum y-corners, then x-corners
                s2 = s2pool.tile([128, R * 2 * C], BF16)
                p4v = p4[:].rearrange("p (i yc cc) -> p i yc cc",
                                      i=R, yc=2, cc=2 * C)
                nc.vector.tensor_tensor(
                    s2[:].rearrange("p (i cc) -> p i cc", i=R, cc=2 * C),
                    p4v[:, :, 0, :], p4v[:, :, 1, :], ALU.add)
                s = spool.tile([128, R * C], BF16)
                s2v = s2[:].rearrange("p (i xc c) -> p i xc c", i=R, xc=2, c=C)
                sv = s[:].rearrange("p (i c) -> p i c", i=R, c=C)
                nc.vector.tensor_tensor(
                    sv, s2v[:, :, 0, :], s2v[:, :, 1, :], ALU.add)
                # transpose to [c, (i, j)]; taps paired on partition halves
                par = k % 2
                if par == 0:
                    st2 = stpool.tile([128, R * 128], BF16)
                for h in range(R // 8):
                    tp = tpps.tile([128, 8 * 128], BF16)
                    for i2 in range(8):
                        i = h * 8 + i2
                        nc.tensor.transpose(
                            tp[par * 64:par * 64 + 64, i2 * 128:(i2 + 1) * 128],
                            sv[:, i, :], ident[:])
                    nc.scalar.copy(
                        st2[par * 64:par * 64 + 64,
                            h * 8 * 128:(h + 1) * 8 * 128],
                        tp[par * 64:par * 64 + 64, :])
                # conv-accumulate: pairs (0,1),(2,3),(4,5),(6,7) full-128
                # contraction; tap 8 contracts 64 alone.
                if k % 2 == 1:
                    kp = k // 2
                    for c4 in range(R * W // 512):
                        nc.tensor.matmul(
                            out_ps[:, c4 * 512:(c4 + 1) * 512],
                            wk2[:, kp * 64:(kp + 1) * 64],
                            st2[:, c4 * 512:(c4 + 1) * 512],
                            start=(kp == 0), stop=False)
                elif k == 8:
                    for c4 in range(R * W // 512):
                        nc.tensor.matmul(
                            out_ps[:, c4 * 512:(c4 + 1) * 512],
                            wkl[:],
                            st2[0:64, c4 * 512:(c4 + 1) * 512],
                            start=False, stop=True)
            ob = obpool.tile([64, R * W], F32)
            nc.scalar.copy(ob[:], out_ps[:])
            nc.sync.dma_start(out_d[:, b * R * W:(b + 1) * R * W], ob[:])

    if not nc.is_finalized():
        nc.finalize()
    return nc


def _prep_shared(x, offset, mask, weight):
    """Per-image R4 tables + weight tiles shared by both cores of an image."""
    # weight is [C_OUT, C_IN, KH, KW] -> [C_OUT, C_IN, K2]
    wf = weight.reshape(C, C, K2)
    # wk2[c + 64*par, kp*64 + o] = W[o, c, 2kp+par] for kp in 0..3
    wk2 = np.zeros((128, 4 * 64), np.float32)
    for kp in range(4):
        for par in range(2):
            k = 2 * kp + par
            wk2[par * 64:(par + 1) * 64, kp * 64:(kp + 1) * 64] = wf[:, :, k].T
    wkl = np.ascontiguousarray(wf[:, :, 8].T)  # [c, o]
    r4s = []
    for n in range(N):
        xp = np.zeros((PH + 1, PW + 1, C), np.float32)
        xp[PAD:PAD + H, PAD:PAD + W, :] = x[n].transpose(1, 2, 0)
        xpb = xp.astype(NPBF16)
        r4 = np.stack([xpb[:PH, :PW], xpb[:PH, 1:PW + 1],
                       xpb[1:PH + 1, :PW], xpb[1:PH + 1, 1:PW + 1]], axis=2)
        r4s.append(np.ascontiguousarray(r4).reshape(-1))
    return r4s, wk2.astype(NPBF16), wkl.astype(NPBF16)


def _prep_core(x, offset, mask, r4s, wk2, wkl, core):
    n, half = core // 2, core % 2
    i0 = half * HI
    offj = np.ascontiguousarray(
        offset[n, :, i0:i0 + HI, :].transpose(2, 0, 1)).reshape(128, 2 * K2 * HI)
    maskj = np.ascontiguousarray(
        mask[n, :, i0:i0 + HI, :].transpose(2, 0, 1)).reshape(128, K2 * HI)

    k = np.arange(K2)
    ki, kj = k // 3, k % 3
    i = np.arange(HI)
    # A(k,i) = (i0 + i + ki - 1 + PAD)*PW + (kj - 1 + PAD), replicated over p
    A = ((i0 + i[None, :] + ki[:, None] - 1 + PAD) * PW
         + kj[:, None] - 1 + PAD).astype(np.float32)  # [K2, HI]
    base = np.broadcast_to(A.reshape(1, K2 * HI), (128, K2 * HI))
    assert A.min() - CLAMP * PW - CLAMP >= 0
    assert A.max() + 127 + CLAMP * PW + CLAMP < NENT

    return {
        "r4": r4s[n],
        "offj": offj,
        "maskj": maskj,
        "base": np.ascontiguousarray(base),
        "j128": np.arange(128, dtype=np.float32).reshape(128, 1),
        "wk2": wk2,
        "wkl": wkl,
        "ident": np.eye(128, dtype=np.float32).astype(NPBF16),
    }


def _run(x, offset, mask, weight, trace=False, trace_kwargs=None):
    x = np.asarray(x, np.float32)
    offset = np.asarray(offset, np.float32)
    mask = np.asarray(mask, np.float32)
    weight = np.asarray(weight, np.float32)

    if "nc" not in _CACHED:
        _CACHED["nc"] = build_nc()
    nc = _CACHED["nc"]

    r4s, wk2, wkl = _prep_shared(x, offset, mask, weight)
    in_maps = [
        _prep_core(x, offset, mask, r4s, wk2, wkl, core) for core in range(8)
    ]
    if trace:
        res = run_bass_kernel_spmd(nc, in_maps, list(range(8)), trace=True,
                                   **(trace_kwargs or {}))
    else:
        res = run_bass_kernel_spmd(nc, in_maps, list(range(8)))
    out = np.empty((N, C, H, W), np.float32)
    for core in range(8):
        n, half = core // 2, core % 2
        out[n, :, half * HI:(half + 1) * HI, :] = (
            res.results[core]["out"].reshape(C, HI, W))
    return out, res


def kernel_traced(x, offset, mask, weight, trace=True, trace_kwargs=None):
    """Like kernel() but runs with NTFF tracing; returns (out, results)."""
    return _run(x, offset, mask, weight, trace=trace, trace_kwargs=trace_kwargs)


def kernel(x, offset, mask, weight):
    out, _ = _run(x, offset, mask, weight, trace=False)
    return out
